# revision 1
# baseline (speedup 1.0000x reference)
"""Trainium2 Bass kernel for nn_Attention_5145370821223.

Computation (per batch b of 16, heads H=6, tokens N=512, dim 78, dh 13):
    qkv = x @ W_qkv ; dots = q k^T / sqrt(13), masked by m_i & m_j
    attn = softmax(dots) * 1.0 + 0.5 * adj * (m_i & m_j)
    y = (attn @ v) @ W_out + b_out

Strategy: data-parallel over batch (2 batches per NeuronCore x 8 cores).
On-core pipeline works in the "transposed" orientation so that the softmax
matrix P^T = exp(dots^T) comes out of the scalar engine already laid out with
the contraction (j) axis on partitions for the attention*V matmuls.

Mask handling (validated bit-for-bit against the jax reference semantics):
  dots'^T[j,i] = m_i*(q_i.k_j/sqrt(dh)) + m_i*(30*m_j - 30)
  - multiplicative m_i on the q-side makes fully-masked rows exactly constant,
    so softmax gives the reference's uniform 1/512 rows;
  - the additive -30*(1-m_j) term (folded into the k-side matmul as an extra
    contraction row) kills masked keys: exp(-30)~9e-14.
  rowsums come free via ones-columns appended to V; the softmax division is
  deferred to the [128, 84] attention output (diag scaling commutes).
  adj term: 0.5*m_i*(adj @ (v*m_j)), shared across heads; adj is transposed
  on the host so its j axis lands on partitions.
"""

import os
import numpy as np
import ml_dtypes

H, DH, DIM = 6, 13, 78
LA, LG = 1.0, 0.5
B, N = 16, 512
SCALE = DH ** -0.5
NEG = 30.0
NCORES = 8
BPC = B // NCORES          # batches per core
NT = N // 128              # 128-token tiles per sequence
HDA = 14                   # dh + 1 (ones column) per head in V_aug
PASS_A = [0, 1, 2, 3]
PASS_B = [4, 5]

_CACHE = {}


# ---------------------------------------------------------------------------
# Workaround: this container's walrus rejects the multi-wait Drain that
# TileContext emits at exit ("Too many sync wait commands"). Split the waits
# into individual wait_ge instructions on the SP engine before a bare drain.
def _apply_tile_patch(tile_mod, ScopedClock):
    def _patched(self, tick_clock, wait_clock):
        nc = self.nc
        drain_inst = nc.sync.drain()
        wait_clock.add_sem_waits(
            drain_inst.ins, ScopedClock({None: tick_clock.global_clock})
        )
        mi = drain_inst.ins
        waits = list(mi.sync_info.on_wait)
        if len(waits) > 1:
            handles = {s.name: s for s in self.sems.allocated().values()}
            engines = [nc.sync, nc.vector, nc.scalar, nc.tensor, nc.gpsimd]
            kept = []
            k = 0
            for w in waits:
                h = handles.get(w.ant_name)
                if h is None:
                    kept.append(w)
                    continue
                engines[k % len(engines)].wait_ge(h, w.wait_value)
                k += 1
            mi.sync_info.on_wait = kept
        nc.all_engine_barrier()
        assert self.sems is not None
        popped = nc._tile_sem_poison_stack.pop()
        assert popped is self._sem_poison
        nc.clear_and_free_semaphores(list(self.sems.allocated().values()))
        nc.all_engine_barrier()

    tile_mod.TileContext._drain_and_barrier = _patched


def _split_waits(nc, mybir):
    """This walrus build only encodes one sem-wait per instruction; hoist
    extra waits onto same-engine NoOps inserted right before the owner."""
    k = 0
    for f in nc.m.functions:
        for bb in f.blocks:
            out = []
            changed = False
            for inst in bb.instructions:
                si = inst.sync_info
                waits = list(si.on_wait) if si is not None else []
                if len(waits) > 1:
                    changed = True
                    for w in waits[:-1]:
                        n = mybir.InstNoOp(name=f"I-wsplit-{k}", ins=[], outs=[])
                        k += 1
                        n.engine = inst.engine
                        n.sync_info = mybir.SyncInfo(on_wait=[w], on_update=[])
                        out.append(n)
                    si.on_wait = [waits[-1]]
                out.append(inst)
            if changed:
                bb.instructions = out


# ---------------------------------------------------------------------------
def _host_weights(W_qkv, W_out, b_out):
    """Rearrange weights into the stationary layouts the kernel uses."""
    W = W_qkv.reshape(DIM, H, 3, DH).astype(np.float32)

    def qk_stack(heads, kind):
        w = np.zeros((80, 128), np.float32)
        for g, h in enumerate(heads):
            c0 = 32 * g
            if kind == "q":
                w[0:DIM, c0:c0 + DH] = W[:, h, 0, :] * SCALE
                w[79, c0 + DH] = 1.0          # ones input row -> m_i after mask mul
            else:
                w[0:DIM, c0:c0 + DH] = W[:, h, 1, :]
                w[78, c0 + DH] = NEG          # mask row  -> +30*m_j
                w[79, c0 + DH] = -NEG         # ones row  -> -30
        return w

    wqa = qk_stack(PASS_A, "q")
    wqb = qk_stack(PASS_B, "q")
    wka = qk_stack(PASS_A, "k")
    wkb = qk_stack(PASS_B, "k")

    wv = np.zeros((80, H * HDA), np.float32)
    for h in range(H):
        wv[0:DIM, h * HDA:h * HDA + DH] = W[:, h, 2, :]

    wo = np.zeros((80, DIM), np.float32)
    wo[0:DIM, :] = W_out.astype(np.float32)
    wo[78, :] = b_out.astype(np.float32)
    # single packed stationary-weights tensor: [80, 4*128 + 84 + 78]
    return np.concatenate([wqa, wqb, wka, wkb, wv, wo], axis=1)


def _build_bass(walrus_patches=True):
    import concourse.bass as bass
    import concourse.mybir as mybir
    import concourse.tile as tile
    from concourse.vector_clock import ScopedClock
    from concourse.masks import make_identity

    if walrus_patches:
        _apply_tile_patch(tile, ScopedClock)

    f32 = mybir.dt.float32
    f32r = mybir.dt.float32r
    bf16 = mybir.dt.bfloat16
    AF = mybir.ActivationFunctionType
    OP = mybir.AluOpType

    nc = bass.Bass()
    WCOLS = 4 * 128 + H * HDA + DIM
    xaug_d = nc.dram_tensor("xaug", [BPC, 80, N], f32r, kind="ExternalInput")
    maskf = nc.dram_tensor("maskf", [BPC, N], f32r, kind="ExternalInput")
    maskc = nc.dram_tensor("maskc", [BPC, N], f32, kind="ExternalInput")
    adjt = nc.dram_tensor("adjt", [BPC, N, N], f32, kind="ExternalInput")
    wall_d = nc.dram_tensor("wall", [80, WCOLS], f32r, kind="ExternalInput")
    yout = nc.dram_tensor("yout", [BPC, N, DIM], f32, kind="ExternalOutput")

    with tile.TileContext(nc) as tc:
        with (
            tc.tile_pool(name="consts", bufs=1) as consts,
            tc.tile_pool(name="bpool", bufs=2) as bpool,
            tc.tile_pool(name="ptpA", bufs=2 * NT) as ptpA,
            tc.tile_pool(name="ptpB", bufs=2 * NT) as ptpB,
            tc.tile_pool(name="spool", bufs=3) as spool,
            tc.tile_pool(name="opool", bufs=8) as opool,
            tc.tile_pool(name="ps_small", bufs=2, space="PSUM") as ps_small,
            tc.tile_pool(name="ps_dotsA", bufs=1, space="PSUM") as ps_dotsA,
            tc.tile_pool(name="ps_dotsB", bufs=1, space="PSUM") as ps_dotsB,
        ):
            # --- constants (weight DMAs are emitted inside phase1(0) after
            # the latency-critical x/mask loads so the DMA queue serves those
            # first) ---
            identity = consts.tile([128, 128], bf16)
            make_identity(nc, identity)
            # tiny warm-up exp: hoists the one-time ~2.7us ACT table load
            # for the exp set into the initial DMA wait instead of the
            # first real exp's critical path
            warm = consts.tile([128, 1], f32, tag="warm")
            nc.vector.memset(warm, 0.0)
            nc.scalar.activation(warm[:], warm[:], AF.Exp)
            ones_col = consts.tile([1, 128], f32r)
            nc.gpsimd.memset(ones_col[:].bitcast(f32), 1.0)
            wall = consts.tile([80, WCOLS], f32r, tag="wall")
            wqa = wall[:, 0:128]
            wqb = wall[:, 128:256]
            wka = wall[:, 256:384]
            wkb = wall[:, 384:512]
            wv = wall[:, 512:512 + H * HDA]
            wo = wall[0:79, 512 + H * HDA:WCOLS]

            def phase1(b):
                # ---- latency-critical loads (mask first: the m_bc chain
                # only needs maskrow + ones_col) ----
                # xTaug rows: 0..77 x^T, 78 mask, 79 ones (host-built)
                maskrow = bpool.tile([1, N], f32r, tag="maskrow")
                nc.sync.dma_start(maskrow[:], maskf[b:b + 1, :])
                xTaug = bpool.tile([80, N], f32r, tag="xTaug")
                nc.sync.dma_start(xTaug[:], xaug_d[b])
                if b == 0:
                    nc.sync.dma_start(wall[:], wall_d[:])

                # ---- mask broadcast [128, N] via PE outer product ----
                psb = ps_small.tile([128, N], f32, tag="ps")
                nc.tensor.matmul(psb[:], ones_col[:], maskrow[:])
                m_bc = bpool.tile([128, N], bf16, tag="m_bc")
                (nc.scalar.copy if b == 0 else nc.vector.tensor_copy)(
                    m_bc[:], psb[:])

                # ---- projections ----
                # batch 0's plain copies go on the scalar engine (idle before
                # the first exp; exp table already loaded by the warm-up);
                # batch 1's stay on DVE (ACT is busy with exps then).
                cp = nc.vector.tensor_copy

                def project(w_sb, out_name, masked, copier=None):
                    ps = ps_small.tile([128, N], f32, tag="ps")
                    nc.tensor.matmul(ps[:], w_sb[:], xTaug[:])
                    dst = bpool.tile([128, N], f32r, tag=out_name)
                    if masked:
                        nc.vector.tensor_tensor(
                            dst[:], ps[:], m_bc[:], op=OP.mult)
                    else:
                        (copier or cp)(dst[:], ps[:])
                    return dst

                qsA = project(wqa, "qsA", True)
                # ksA's input is ready before the first exp's, so its copy can
                # ride the otherwise-idle scalar engine without blocking exps
                ksA = project(wka, "ksA", False,
                              copier=nc.scalar.copy if b == 0 else None)

                # ---- dots^T + exp; pass A (heads 0-3) / pass B (heads 4,5)
                # use separate psum pools so chunk jt+1's matmuls overlap the
                # exp of chunk jt and the scalar engine never starves ----
                ptsA, ptsB = [], []
                qsB = ksB = None
                for jt in range(NT):
                    jsl = slice(jt * 128, (jt + 1) * 128)
                    psdA = ps_dotsA.tile([128, 4, N], f32, tag="psdA")
                    for g in range(4):
                        c = 32 * g
                        nc.tensor.matmul(
                            psdA[:, g, :], ksA[c:c + DH + 1, jsl],
                            qsA[c:c + DH + 1, :], tile_position=(c, 0))
                    ptA = ptpA.tile([128, 4, N], bf16, tag="ptA")
                    nc.scalar.activation(
                        ptA[:].rearrange("p h n -> p (h n)"),
                        psdA[:].rearrange("p h n -> p (h n)"), AF.Exp)
                    ptsA.append(ptA)

                    if jt == 0:
                        qsB = project(wqb, "qsB", True)
                        ksB = project(wkb, "ksB", False)

                    psdB = ps_dotsB.tile([128, 2, N], f32, tag="psdB")
                    for g in range(2):
                        c = 32 * g
                        nc.tensor.matmul(
                            psdB[:, g, :], ksB[c:c + DH + 1, jsl],
                            qsB[c:c + DH + 1, :], tile_position=(c, 0))
                    ptB = ptpB.tile([128, 2, N], bf16, tag="ptB")
                    nc.scalar.activation(
                        ptB[:].rearrange("p h n -> p (h n)"),
                        psdB[:].rearrange("p h n -> p (h n)"), AF.Exp)
                    ptsB.append(ptB)

                mcol = bpool.tile([128, NT], f32, tag="mcol")
                nc.sync.dma_start(
                    mcol[:], maskc[b].rearrange("(t p) -> p t", p=128))

                # ---- V projection + V natural + masked V (phase-2 inputs) --
                psv = ps_small.tile([H * HDA, N], f32, tag="ps")
                nc.tensor.matmul(psv[:], wv[:], xTaug[:])
                vT = bpool.tile([H * HDA, N], bf16, tag="vT")
                cp(vT[:], psv[:])
                va = bpool.tile([128, NT, H * HDA], bf16, tag="va")
                vm = bpool.tile([128, NT, H * DH], f32, tag="vm")
                for t in range(NT):
                    psvt = ps_small.tile([128, H * HDA], bf16, tag="ps")
                    nc.tensor.transpose(
                        psvt[:], vT[:, t * 128:(t + 1) * 128],
                        identity[0:H * HDA, 0:H * HDA])
                    nc.vector.tensor_copy(va[:, t, :], psvt[:])
                    # masked V for the adjacency term, in f32 from the psum
                    # (keeps the dominant adj@V path at full precision)
                    nc.vector.tensor_scalar(
                        vm[:, t, :].rearrange("p (h c) -> p h c", c=DH),
                        psvt[:].rearrange("p (h c) -> p h c", c=HDA)[:, :, 0:DH],
                        mcol[:, t:t + 1], LG,
                        op0=OP.mult, op1=OP.mult)
                # ones columns (rowsum trick), after the copies (WAW ordered)
                nc.gpsimd.memset(
                    va[:].rearrange("p t (h c) -> p t h c", c=HDA)[:, :, :, DH:HDA],
                    1.0)

                # adjacency load is only needed by phase 2; emitted last (and
                # on the SWDGE queue) so it overlaps the exp phase.
                adjs = bpool.tile([128, NT, N], f32, tag="adjs")
                nc.sync.dma_start(
                    adjs[:], adjt[b].rearrange("(t p) i -> p t i", p=128))
                return dict(ptsA=ptsA, ptsB=ptsB, va=va, vm=vm, adjs=adjs,
                            mcol=mcol)

            def phase2(b, st):
                ptsA, ptsB, va, vm, adjs, mcol = (
                    st["ptsA"], st["ptsB"], st["va"], st["vm"], st["adjs"],
                    st["mcol"])
                outT = bpool.tile([79, N], f32r, tag="outT")
                ysb = bpool.tile([128, NT, DIM], f32, tag="ysb")
                # attn buffer: col 78 is a ones column so the transpose
                # carries the bias row for the output projection.
                attn = bpool.tile([128, NT, 79], bf16, tag="attn")
                nc.gpsimd.memset(attn[:, :, 78:79], 1.0)
                # AV + adj accumulation in psum, split into jt halves so the
                # psum slot is only held for ~2 exp periods; the halves are
                # summed in sbuf during the combine.
                osbs = []
                for it in range(4):
                    isl = slice(it * 128, (it + 1) * 128)
                    for half in range(1):
                        jts = (0, 1, 2, 3) if half == 0 else ()
                        pso = ps_small.tile([128, 162], f32, tag="ps")
                        # one accumulation group for the whole pso bank:
                        # start only on the first matmul, stop on the last
                        # (interleaved per-region groups corrupt each other --
                        # the start flag's zero region is bank-granular).
                        last = len(jts) - 1
                        for jj, jt in enumerate(jts):
                            for h in range(4):
                                nc.tensor.matmul(
                                    pso[:, h * HDA:(h + 1) * HDA],
                                    ptsA[jt][:, h, isl],
                                    va[:, jt, h * HDA:(h + 1) * HDA],
                                    start=(jj == 0 and h == 0), stop=False,
                                    skip_group_check=True)
                            for h in range(4, H):
                                nc.tensor.matmul(
                                    pso[:, h * HDA:(h + 1) * HDA],
                                    ptsB[jt][:, h - 4, isl],
                                    va[:, jt, h * HDA:(h + 1) * HDA],
                                    start=False, stop=False,
                                    skip_group_check=True)
                            nc.tensor.matmul(
                                pso[:, 84:84 + H * DH],
                                adjs[:, jt, isl],
                                vm[:, jt, :],
                                start=False, stop=(jj == last),
                                skip_group_check=True)
                        osb = opool.tile([128, 162], f32, tag="osb")
                        (nc.scalar.copy if b == BPC - 1
                         else nc.vector.tensor_copy)(osb[:], pso[:])
                        osbs.append(osb)

                for it in range(4):
                    isl = slice(it * 128, (it + 1) * 128)
                    osb = osbs[it]
                    pv_heads = osb[:, 0:84].rearrange("p (h c) -> p h c", c=HDA)
                    rs6 = spool.tile([128, H], f32, tag="rs6")
                    nc.vector.reciprocal(
                        rs6[:].unsqueeze(2), pv_heads[:, :, DH:HDA])
                    attn_pv = spool.tile([128, H * DH], f32, tag="attn_pv")
                    nc.gpsimd.tensor_tensor(
                        attn_pv[:].rearrange("p (h c) -> p h c", c=DH),
                        pv_heads[:, :, 0:DH],
                        rs6[:].unsqueeze(2).broadcast_to([128, H, DH]),
                        op=OP.mult)
                    nc.vector.scalar_tensor_tensor(
                        attn[:, it, 0:H * DH], osb[:, 84:84 + H * DH],
                        mcol[:, it:it + 1],
                        attn_pv[:], op0=OP.mult, op1=OP.add)
                    # transpose + output projection (bias via ones column)
                    tailcp = nc.vector.tensor_copy
                    psa = ps_small.tile([79, 128], bf16, tag="ps")
                    nc.tensor.transpose(psa[:], attn[:, it, :], identity[:])
                    tailcp(outT[:, isl], psa[:])
                    psy = ps_small.tile([128, DIM], f32, tag="ps")
                    nc.tensor.matmul(psy[:], outT[:, isl], wo[:])
                    tailcp(ysb[:, it, :], psy[:])
                    nc.sync.dma_start(
                        yout[b].rearrange("(t p) f -> p t f", p=128)[:, it, :],
                        ysb[:, it, :])

            # batch-level software pipeline: both batches' setup/dots/exp are
            # emitted before either batch's AV/output phase, so the scalar
            # engine (the bottleneck: fused exps) runs back-to-back while
            # PE/DVE fill in AV and output work underneath.
            states = [phase1(b) for b in range(BPC)]
            for b in range(BPC):
                phase2(b, states[b])

    if walrus_patches:
        _split_waits(nc, mybir)
    return nc


def _prep_inputs(x, mask, adjacency_mat, W_qkv, W_out, b_out):
    x = np.asarray(x, np.float32)
    maskf = np.ascontiguousarray(np.asarray(mask, np.float32))
    adj = np.asarray(adjacency_mat, np.float32)
    adjt = np.ascontiguousarray(adj.transpose(0, 2, 1))
    wall = _host_weights(
        np.asarray(W_qkv, np.float32), np.asarray(W_out, np.float32),
        np.asarray(b_out, np.float32))
    # xaug: rows 0..77 x^T, row 78 mask, row 79 ones (built on host)
    xaug = np.zeros((B, 80, N), np.float32)
    xaug[:, 0:DIM, :] = x.transpose(0, 2, 1)
    xaug[:, 78, :] = maskf.astype(np.float32)
    xaug[:, 79, :] = 1.0
    in_maps = []
    for c in range(NCORES):
        s = slice(c * BPC, (c + 1) * BPC)
        in_maps.append({
            "xaug": np.ascontiguousarray(xaug[s]),
            "maskf": np.ascontiguousarray(maskf[s].astype(np.float32)),
            "maskc": np.ascontiguousarray(maskf[s].astype(np.float32)),
            "adjt": np.ascontiguousarray(adjt[s]),
            "wall": wall,
        })
    return in_maps


LAST_EXEC_NS = None
LAST_RESULT = None


def kernel(x, mask, adjacency_mat, W_qkv, W_out, b_out):
    global LAST_EXEC_NS, LAST_RESULT
    from concourse.bass_utils import run_bass_kernel_spmd

    if "nc" not in _CACHE:
        _CACHE["nc"] = _build_bass()
    nc = _CACHE["nc"]

    in_maps = _prep_inputs(x, mask, adjacency_mat, W_qkv, W_out, b_out)
    trace = bool(int(os.environ.get("KERNEL_TRACE", "0")))
    res = run_bass_kernel_spmd(
        nc, in_maps, core_ids=list(range(NCORES)), trace=trace)
    LAST_EXEC_NS = res.exec_time_ns
    LAST_RESULT = res
    y = np.concatenate([res.results[c]["yout"] for c in range(NCORES)], axis=0)
    return np.ascontiguousarray(y.astype(np.float32))



# revision 36
# speedup vs baseline: 1.2171x; 1.2171x over previous
"""Trainium2 Bass kernel for nn_Attention_5145370821223.

Computation (per batch b of 16, heads H=6, tokens N=512, dim 78, dh 13):
    qkv = x @ W_qkv ; dots = q k^T / sqrt(13), masked by m_i & m_j
    attn = softmax(dots) * 1.0 + 0.5 * adj * (m_i & m_j)
    y = (attn @ v) @ W_out + b_out

Strategy: data-parallel over batch (2 batches per NeuronCore x 8 cores).
The softmax exponentials dominate (6*512*512 elems/batch); they are split
across BOTH the scalar engine (exact Exp activation) and the vector engine
(one-instruction Schraudolph fast-exp: int16(128*log2e*x + 16250.496)
bitcast to bf16, max ~3% elementwise rel err which cancels between softmax
numerator and denominator; end-to-end rel err ~2.5e-3, gate is 2e-2).

Mask handling (same algebra as v1, but folded into host-side data):
  xq rows 0..77 hold x^T pre-multiplied by m_i, row 79 = m_i, so the q
  projection emits m_i*q and a m_i column with no on-chip mask multiply.
  xkv rows 0..77 = x^T, row 78 = m_j, row 79 = 1; the k weights put
  +-30 in the paired column so dots'^T[j,i] = m_i*qk + m_i*(30 m_j - 30).
  Fully-masked rows become exp(0)=1 -> uniform 1/512 (reference semantics);
  masked keys get exp(-30)~9e-14. Rowsums ride ones-columns in V.
  adj term: (0.5*adj)@(m_j*v) with 0.5 folded into the host adj transpose.

Pipeline per batch: projections (1 psum bank), then per jt in 0..3 the
dots matmuls fill pass-A (heads 0-2, 3 banks) and pass-B (heads 3-5,
3 banks) psum regions, exp'd chunk-by-chunk by ACT/DVE per EXP_SCHED.
Attention@V accumulates progressively in a single psum bank as it-pair x
jt-half tiles so only the last half remains after the final exp (short
tail). Output: combine, PE transpose, out-proj, bf16 DMA out (host casts).
"""

import os
import numpy as np
import ml_dtypes

H, DH, DIM = 6, 13, 78
LA, LG = 1.0, 0.5
B, N = 16, 512
SCALE = DH ** -0.5
NEG = 30.0
NCORES = 8
BPC = B // NCORES          # batches per core
NT = N // 128              # 128-token tiles per sequence
HDA = 14                   # dh + 1 (ones column) per head in V_aug
HPP = 3                    # heads per pass (A: 0-2, B: 3-5)
LOG2E = 1.4426950408889634
FE_A = 128.0 * LOG2E       # fast-exp scale
FE_B = 127.0 * 128.0 - 128.0 * 0.043   # fast-exp bias (tuned c)
ATTN_W = 79                # attn cols per it (78 + ones col for bias)
PE_WARMUP = 12             # identity transposes to ramp the PE p-state

# exp engine schedule: (b, jt, ci) -> "act" | "dve", where chunk ci covers
# heads (2ci, 2ci+1). ACT is cheaper per element (0.833 vs 1.042 ns) so it
# gets the majority; DVE runs the fast-exp on its share.
EXP_SCHED = {}
for _b in range(BPC):
    for _jt in range(NT):
        for _ci in range(3):
            EXP_SCHED[(_b, _jt, _ci)] = "dve" if _ci == 2 else "act"
# trace-tuned extras: shifts exp work toward DVE where its queue has slack
EXP_SCHED[(1, 0, 1)] = "dve"
EXP_SCHED[(0, 1, 1)] = "dve"

# engine for each drain / post op, per batch: tuned against the trace.
DRAIN_SCHED = {
    ("q0", 0): "act", ("k0", 0): "dve", ("q1", 0): "dve", ("k1", 0): "dve",
    ("va", 0): "dve",
    ("q0", 1): "act", ("k0", 1): "act", ("q1", 1): "dve", ("k1", 1): "act",
    ("va", 1): "dve",
    ("pv0", 0): "dve", ("pv1", 0): "dve",
    ("pv0", 1): "act", ("pv1", 1): "act",
    ("outT", 0): "dve", ("outT", 1): "act",
    ("ysb", 0): "dve", ("ysb", 1): "act",
}

_CACHE = {}


# ---------------------------------------------------------------------------
# Workaround: this container's walrus rejects the multi-wait Drain that
# TileContext emits at exit ("Too many sync wait commands"). Split the waits
# into individual wait_ge instructions on the SP engine before a bare drain.
def _apply_tile_patch(tile_mod, ScopedClock):
    def _patched(self, tick_clock, wait_clock):
        nc = self.nc
        drain_inst = nc.sync.drain()
        wait_clock.add_sem_waits(
            drain_inst.ins, ScopedClock({None: tick_clock.global_clock})
        )
        mi = drain_inst.ins
        waits = list(mi.sync_info.on_wait)
        if len(waits) > 1:
            handles = {s.name: s for s in self.sems.allocated().values()}
            engines = [nc.sync, nc.vector, nc.scalar, nc.tensor, nc.gpsimd]
            kept = []
            k = 0
            for w in waits:
                h = handles.get(w.ant_name)
                if h is None:
                    kept.append(w)
                    continue
                engines[k % len(engines)].wait_ge(h, w.wait_value)
                k += 1
            mi.sync_info.on_wait = kept
        nc.all_engine_barrier()
        assert self.sems is not None
        popped = nc._tile_sem_poison_stack.pop()
        assert popped is self._sem_poison
        # no barrier after the sem clears: they ride the SP stream, which
        # the runtime waits out anyway before the NEFF completes
        nc.clear_and_free_semaphores(list(self.sems.allocated().values()))

    tile_mod.TileContext._drain_and_barrier = _patched


def _split_waits(nc, mybir):
    """This walrus build only encodes one sem-wait per instruction; hoist
    extra waits onto same-engine NoOps inserted right before the owner."""
    k = 0
    for f in nc.m.functions:
        for bb in f.blocks:
            out = []
            changed = False
            for inst in bb.instructions:
                si = inst.sync_info
                waits = list(si.on_wait) if si is not None else []
                if len(waits) > 1:
                    changed = True
                    for w in waits[:-1]:
                        n = mybir.InstNoOp(name=f"I-wsplit-{k}", ins=[], outs=[])
                        k += 1
                        n.engine = inst.engine
                        n.sync_info = mybir.SyncInfo(on_wait=[w], on_update=[])
                        out.append(n)
                    si.on_wait = [waits[-1]]
                out.append(inst)
            if changed:
                bb.instructions = out


# ---------------------------------------------------------------------------
def _host_weights(W_qkv, W_out, b_out):
    """Stationary weights, bf16: [wqA wqB wkA wkB | wv | wo]."""
    W = W_qkv.reshape(DIM, H, 3, DH).astype(np.float32)

    def qk_stack(heads, kind):
        w = np.zeros((80, 128), np.float32)
        for g, h in enumerate(heads):
            c0 = 32 * g
            if kind == "q":
                w[0:DIM, c0:c0 + DH] = W[:, h, 0, :] * SCALE
                w[79, c0 + DH] = 1.0          # xq row 79 = m_i -> col is m_i
            else:
                w[0:DIM, c0:c0 + DH] = W[:, h, 1, :]
                w[78, c0 + DH] = NEG          # xkv row 78 = m_j -> +30*m_j
                w[79, c0 + DH] = -NEG         # xkv row 79 = 1   -> -30
        return w

    pa, pb = [0, 1, 2], [3, 4, 5]
    wqa, wqb = qk_stack(pa, "q"), qk_stack(pb, "q")
    wka, wkb = qk_stack(pa, "k"), qk_stack(pb, "k")

    wv = np.zeros((80, H * HDA), np.float32)
    for h in range(H):
        wv[0:DIM, h * HDA:h * HDA + DH] = W[:, h, 2, :]

    wo = np.zeros((80, DIM), np.float32)
    wo[0:DIM, :] = W_out.astype(np.float32)
    wo[78, :] = b_out.astype(np.float32)      # attn ones col -> bias
    full = np.concatenate([wqa, wqb, wka, wkb, wv, wo], axis=1)
    return full.astype(ml_dtypes.bfloat16)


WCOLS = 4 * 128 + H * HDA + DIM


def _build_bass(walrus_patches=True):
    import concourse.bass as bass
    import concourse.mybir as mybir
    import concourse.tile as tile
    from concourse.vector_clock import ScopedClock
    from concourse.masks import make_identity

    if walrus_patches:
        _apply_tile_patch(tile, ScopedClock)

    f32 = mybir.dt.float32
    bf16 = mybir.dt.bfloat16
    i16 = mybir.dt.int16
    AF = mybir.ActivationFunctionType
    OP = mybir.AluOpType

    nc = bass.Bass()
    # xall: [xq cols 0:N | xkv cols N:2N]; adjm: adj tiles + mcol col per tile
    xall_d = nc.dram_tensor("xall", [BPC, 80, 2 * N], bf16, kind="ExternalInput")
    adjm_d = nc.dram_tensor("adjm", [BPC, 128, NT * (N + 1)], bf16,
                            kind="ExternalInput")
    wall_d = nc.dram_tensor("wall", [80, WCOLS], bf16, kind="ExternalInput")
    yout = nc.dram_tensor("yout", [BPC, N, DIM], bf16, kind="ExternalOutput")

    with tile.TileContext(nc) as tc:
        with (
            tc.tile_pool(name="consts", bufs=1) as consts,
            tc.tile_pool(name="bpool", bufs=2) as bpool,
            tc.tile_pool(name="ptp", bufs=2 * NT) as ptp,
            tc.tile_pool(name="spool", bufs=4) as spool,
            tc.tile_pool(name="opool", bufs=2) as opool,
            tc.tile_pool(name="ps_d0", bufs=1, space="PSUM") as ps_d0,
            tc.tile_pool(name="ps_d1", bufs=1, space="PSUM") as ps_d1,
            tc.tile_pool(name="ps_d2", bufs=1, space="PSUM") as ps_d2,
            tc.tile_pool(name="ps_av0", bufs=1, space="PSUM") as ps_av0,
            tc.tile_pool(name="ps_av1", bufs=1, space="PSUM") as ps_av1,
        ):
            PSD = [ps_d0, ps_d1, ps_d2]
            PSA = [ps_av0, ps_av1]
            identity = consts.tile([128, 128], bf16)
            make_identity(nc, identity)
            # warm-up exp hoists the one-time ACT table load off the
            # first real exp's critical path
            warm = consts.tile([128, 1], f32, tag="warm")
            nc.vector.memset(warm, 0.0)
            nc.scalar.activation(warm[:], warm[:], AF.Exp)
            wall = consts.tile([80, WCOLS], bf16, tag="wall")
            nc.sync.dma_start(wall[:], wall_d[:])
            wq = [wall[:, 0:128], wall[:, 128:256]]
            wk = [wall[:, 256:384], wall[:, 384:512]]
            wv = wall[:, 512:512 + H * HDA]
            wo = wall[0:79, 512 + H * HDA:WCOLS]
            # PE p-state warm-up: keep the tensor engine continuously busy
            # from t~0 so real matmuls run at the ramped rate. P2 is the
            # last pool needed by real work.
            for wi in range(PE_WARMUP):
                pw = ps_d2.tile([128, 128], bf16, tag="psd2")
                nc.tensor.transpose(pw[:], identity[:], identity[:])

            DR = {"act": nc.scalar.copy, "dve": nc.vector.tensor_copy}

            def dma_in(b):
                xall = bpool.tile([80, 2 * N], bf16, tag="xall")
                nc.sync.dma_start(xall[:], xall_d[b])
                adjm = bpool.tile([128, NT, N + 1], bf16, tag="adjm")
                nc.sync.dma_start(adjm[:], adjm_d[b].rearrange(
                    "p (t i) -> p t i", t=NT))
                mcol = spool.tile([128, NT], f32, tag="mcol")
                nc.vector.tensor_copy(mcol[:], adjm[:, :, N])
                return dict(xq=xall[:, 0:N], xkv=xall[:, N:2 * N],
                            adjs=adjm[:, :, 0:N], mcol=mcol,
                            qs=[None, None], ks=[None, None])

            def proj_qk(b, st, which, pool):
                """One q or k projection ('q0','k0','q1','k1') via `pool`."""
                kind, s = which[0], int(which[1])
                w = (wq if kind == "q" else wk)[s]
                x = st["xq"] if kind == "q" else st["xkv"]
                ps = pool.tile([128, N], f32, tag=pool.name.replace("ps_", "ps"))
                nc.tensor.matmul(ps[:], w, x)
                sb = bpool.tile([128, N], bf16, tag=which)
                DR[DRAIN_SCHED[(which, b)]](sb[:], ps[:])
                (st["qs"] if kind == "q" else st["ks"])[s] = sb

            def proj_v(b, st, pool):
                psv = pool.tile([128, NT * H * HDA], f32, tag=pool.name.replace("ps_", "ps"))
                for t in range(NT):
                    nc.tensor.matmul(
                        psv[:, t * H * HDA:(t + 1) * H * HDA],
                        st["xkv"][0:DIM, t * 128:(t + 1) * 128], wv[0:DIM, :],
                        start=(t == 0), stop=(t == NT - 1),
                        skip_group_check=True)
                va = bpool.tile([128, NT, H * HDA], bf16, tag="va")
                DR[DRAIN_SCHED[("va", b)]](
                    va[:].rearrange("p t c -> p (t c)"), psv[:])
                vm = bpool.tile([128, NT, H * DH], bf16, tag="vm")
                for t in range(NT):
                    nc.gpsimd.tensor_tensor(
                        vm[:, t, :].rearrange("p (h c) -> p h c", c=DH),
                        va[:, t, :].rearrange("p (h c) -> p h c", c=HDA)[:, :, 0:DH],
                        st["mcol"][:, t:t + 1].unsqueeze(2).broadcast_to(
                            [128, H, DH]),
                        op=OP.mult)
                nc.gpsimd.memset(
                    va[:].rearrange("p t (h c) -> p t h c", c=HDA)[:, :, :, DH:HDA],
                    1.0)
                st["va"], st["vm"] = va, vm

            def emit_dots(b, jt, st, cis=(0, 1, 2), pts=None):
                """Three 2-head chunks for token tile jt; chunk ci owns
                pool ci (ACT: P0/P1, DVE: P2) so each engine's next chunk
                is always pre-fillable."""
                jsl = slice(jt * 128, (jt + 1) * 128)
                if pts is None:
                    pts = []
                for ci in cis:
                    psd = PSD[ci].tile([128, 2, N], f32, tag=f"psd{ci}")
                    for k in range(2):
                        h = 2 * ci + k
                        t, g = divmod(h, HPP)
                        c = 32 * g
                        nc.tensor.matmul(
                            psd[:, k, :], st["ks"][t][c:c + DH + 1, jsl],
                            st["qs"][t][c:c + DH + 1, :],
                            tile_position=(c, 0))
                    eng = EXP_SCHED[(b, jt, ci)]
                    if eng == "act":
                        pt = ptp.tile([128, 2, N], bf16, tag=f"pt{ci}")
                        nc.scalar.activation(
                            pt[:].rearrange("p h n -> p (h n)"),
                            psd[:].rearrange("p h n -> p (h n)"), AF.Exp)
                    else:
                        pt = ptp.tile([128, 2, N], i16, tag=f"pt{ci}")
                        nc.vector.tensor_scalar(
                            pt[:].rearrange("p h n -> p (h n)"),
                            psd[:].rearrange("p h n -> p (h n)"),
                            FE_A, FE_B, op0=OP.mult, op1=OP.add)
                    pts.append(pt)
                return pts

            def emit_av_jt(b, st, psos, pts_jt, jt, parts=("adj", 0, 1, 2)):
                """Accumulate this jt's attention@V / adj@vm contributions
                into the two it-pair psum banks. `parts` selects which
                chunk's head-matmuls (0/1/2) or the adjacency ("adj") are
                emitted, so each piece only queues behind the exp it needs.
                The psum group start is the first jt0 adj matmul and the
                stop is the last jt3 ci2 matmul (same-engine in-order
                execution keeps the zeroing first and the stop last)."""
                for part in parts:
                    for itp in range(2):
                        pso = psos[itp]
                        for ki in range(2):
                            it = 2 * itp + ki
                            isl = slice(it * 128, (it + 1) * 128)
                            if part == "adj":
                                nc.tensor.matmul(
                                    pso[:, ki, H * HDA:],
                                    st["adjs"][:, jt, isl],
                                    st["vm"][:, jt, :],
                                    start=(jt == 0 and ki == 0),
                                    stop=False, skip_group_check=True)
                                continue
                            for hh in range(2):
                                h = 2 * part + hh
                                pt = pts_jt[part]
                                ptv = pt[:, hh, isl]
                                if pt.dtype != bf16:
                                    ptv = ptv.bitcast(bf16)
                                nc.tensor.matmul(
                                    pso[:, ki, h * HDA:(h + 1) * HDA], ptv,
                                    st["va"][:, jt, h * HDA:(h + 1) * HDA],
                                    start=False,
                                    stop=(jt == NT - 1 and part == 2
                                          and ki == 1 and hh == 1),
                                    skip_group_check=True)

            def emit_post(b, st, psos, attn):
                """drain AV psum; softmax divide (Pool) + adj add -> attn."""
                for itp in range(2):
                    pso = psos[itp]
                    pv = opool.tile([128, 2, H * HDA + H * DH], bf16,
                                    tag=f"pv{itp}")
                    DR[DRAIN_SCHED[(f"pv{itp}", b)]](
                        pv[:].rearrange("p k c -> p (k c)"),
                        pso[:].rearrange("p k c -> p (k c)"))
                    pvh = pv[:, :, 0:H * HDA].rearrange(
                        "p k (h c) -> p k h c", c=HDA)
                    rs = spool.tile([128, 2, H], f32, tag=f"rs{itp}")
                    nc.vector.reciprocal(
                        rs[:].unsqueeze(3), pvh[:, :, :, DH:HDA])
                    t1 = spool.tile([128, 2, H * DH], bf16, tag=f"t1{itp}")
                    eng = (nc.gpsimd if itp == 0 else nc.vector)
                    eng.tensor_tensor(
                        t1[:].rearrange("p k (h c) -> p k h c", c=DH),
                        pvh[:, :, :, 0:DH],
                        rs[:].unsqueeze(3).broadcast_to([128, 2, H, DH]),
                        op=OP.mult)
                    for ki in range(2):
                        it = 2 * itp + ki
                        nc.vector.scalar_tensor_tensor(
                            attn[:, it, 0:H * DH],
                            pv[:, ki, H * HDA:], st["mcol"][:, it:it + 1],
                            t1[:, ki, :], op0=OP.mult, op1=OP.add)

            def emit_out(b, st, attn):
                """transpose + output projection + store, all 4 its."""
                outT = bpool.tile([79, N], bf16, tag="outT")
                ysb = opool.tile([128, NT, DIM], bf16, tag="ysb")
                # psy lives in PSD1; psa ping-pongs PSD0/PSD2 (no shared
                # pool, so interleaving transposes and psy matmuls is safe)
                psy = PSD[1].tile([128, NT * DIM], f32, tag="psd1")
                for it in range(NT):
                    isl = slice(it * 128, (it + 1) * 128)
                    pool = PSD[2 * (it % 2)]
                    psa = pool.tile([79, 128], bf16,
                                    tag=pool.name.replace("ps_", "ps"))
                    nc.tensor.transpose(
                        psa[:], attn[:, it, 0:79], identity[:])
                    DR[DRAIN_SCHED[("outT", b)]](outT[:, isl], psa[:])
                    nc.tensor.matmul(
                        psy[:, it * DIM:(it + 1) * DIM],
                        outT[:, isl], wo[:],
                        start=(it == 0), stop=(it == NT - 1),
                        skip_group_check=True)
                DR[DRAIN_SCHED[("ysb", b)]](
                    ysb[:].rearrange("p t f -> p (t f)"), psy[:])
                nc.sync.dma_start(
                    yout[b].rearrange("(t p) f -> p t f", p=128), ysb[:])

            def new_attn():
                attn = bpool.tile([128, NT, ATTN_W], bf16, tag="attn")
                nc.gpsimd.memset(attn[:, :, DIM:ATTN_W], 1.0)
                return attn

            def new_psos():
                return [PSA[itp].tile([128, 2, H * HDA + H * DH], f32,
                                      tag=f"av{itp}", name=f"pso{itp}")
                        for itp in range(2)]

            # ---- emission schedule (per-engine queues execute in this
            # order; chosen so the exp streams never sit behind later-dep
            # work in their queues) ----
            st0 = dma_in(0)
            proj_qk(0, st0, "q0", PSD[0])
            proj_qk(0, st0, "k0", PSD[1])
            proj_qk(0, st0, "q1", PSD[2])
            psos0 = new_psos()
            attn0 = new_attn()
            p00 = emit_dots(0, 0, st0, cis=(0,))
            proj_qk(0, st0, "k1", PSD[1])
            emit_dots(0, 0, st0, cis=(1,), pts=p00)
            proj_v(0, st0, PSD[2])
            emit_dots(0, 0, st0, cis=(2,), pts=p00)
            p01 = emit_dots(0, 1, st0)
            emit_av_jt(0, st0, psos0, p00, 0)
            st1 = dma_in(1)
            proj_qk(1, st1, "q0", PSD[0])
            proj_qk(1, st1, "k0", PSD[1])
            proj_qk(1, st1, "q1", PSD[2])
            p02 = emit_dots(0, 2, st0)
            emit_av_jt(0, st0, psos0, p01, 1)
            proj_qk(1, st1, "k1", PSD[0])
            proj_v(1, st1, PSD[1])
            p03 = emit_dots(0, 3, st0)
            emit_av_jt(0, st0, psos0, p02, 2)
            pts1 = [emit_dots(1, 0, st1)]
            emit_av_jt(0, st0, psos0, p03, 3)
            emit_post(0, st0, psos0, attn0)
            pts1.append(emit_dots(1, 1, st1))
            pts1.append(emit_dots(1, 2, st1))
            psos1 = new_psos()
            attn1 = new_attn()
            emit_av_jt(1, st1, psos1, pts1[0], 0)
            emit_av_jt(1, st1, psos1, pts1[1], 1)
            pts1.append(emit_dots(1, 3, st1))
            emit_av_jt(1, st1, psos1, pts1[2], 2)
            emit_out(0, st0, attn0)
            emit_av_jt(1, st1, psos1, pts1[3], 3)
            emit_post(1, st1, psos1, attn1)
            emit_out(1, st1, attn1)

    if walrus_patches:
        _split_waits(nc, mybir)
    return nc


def _prep_inputs(x, mask, adjacency_mat, W_qkv, W_out, b_out):
    x = np.asarray(x, np.float32)
    maskf = np.ascontiguousarray(np.asarray(mask, np.float32))
    adj = np.asarray(adjacency_mat, np.float32)
    wall = _host_weights(
        np.asarray(W_qkv, np.float32), np.asarray(W_out, np.float32),
        np.asarray(b_out, np.float32))
    xt = x.transpose(0, 2, 1)                      # [B, DIM, N]
    # xall: [xq | xkv] side by side (one DMA per batch)
    xall = np.zeros((B, 80, 2 * N), np.float32)
    xall[:, 0:DIM, 0:N] = xt * maskf[:, None, :]   # pre-masked x^T
    xall[:, 79, 0:N] = maskf
    xall[:, 0:DIM, N:2 * N] = xt
    xall[:, 78, N:2 * N] = maskf
    xall[:, 79, N:2 * N] = 1.0
    xall = xall.astype(ml_dtypes.bfloat16)
    # adjm: per 128-row tile, [adj^T * 0.5 tile cols | mask col]
    adjt = (adj * LG).transpose(0, 2, 1)           # [B, j, i]
    adjm = np.zeros((B, 128, NT, N + 1), np.float32)
    adjm[:, :, :, 0:N] = adjt.reshape(B, NT, 128, N).transpose(0, 2, 1, 3)
    adjm[:, :, :, N] = maskf.reshape(B, NT, 128).transpose(0, 2, 1)
    adjm = adjm.reshape(B, 128, NT * (N + 1)).astype(ml_dtypes.bfloat16)
    in_maps = []
    for c in range(NCORES):
        s = slice(c * BPC, (c + 1) * BPC)
        in_maps.append({
            "xall": np.ascontiguousarray(xall[s]),
            "adjm": np.ascontiguousarray(adjm[s]),
            "wall": wall,
        })
    return in_maps


LAST_EXEC_NS = None
LAST_RESULT = None


def kernel(x, mask, adjacency_mat, W_qkv, W_out, b_out):
    global LAST_EXEC_NS, LAST_RESULT
    from concourse.bass_utils import run_bass_kernel_spmd

    if "nc" not in _CACHE:
        _CACHE["nc"] = _build_bass()
    nc = _CACHE["nc"]

    in_maps = _prep_inputs(x, mask, adjacency_mat, W_qkv, W_out, b_out)
    trace = bool(int(os.environ.get("KERNEL_TRACE", "0")))
    res = run_bass_kernel_spmd(
        nc, in_maps, core_ids=list(range(NCORES)), trace=trace)
    LAST_EXEC_NS = res.exec_time_ns
    LAST_RESULT = res
    y = np.concatenate(
        [np.asarray(res.results[c]["yout"]).astype(np.float32)
         for c in range(NCORES)], axis=0)
    return np.ascontiguousarray(y)


# revision 46
# speedup vs baseline: 1.2497x; 1.0268x over previous
"""Trainium2 Bass kernel for nn_Attention_5145370821223.

Computation (per batch b of 16, heads H=6, tokens N=512, dim 78, dh 13):
    qkv = x @ W_qkv ; dots = q k^T / sqrt(13), masked by m_i & m_j
    attn = softmax(dots) * 1.0 + 0.5 * adj * (m_i & m_j)
    y = (attn @ v) @ W_out + b_out

Strategy: data-parallel over batch (2 batches per NeuronCore x 8 cores).
The softmax exponentials dominate (6*512*512 elems/batch); they are split
across BOTH the scalar engine (exact Exp activation) and the vector engine
(one-instruction Schraudolph fast-exp: int16(128*log2e*x + 16250.496)
bitcast to bf16, max ~3% elementwise rel err which cancels between softmax
numerator and denominator; end-to-end rel err ~2.5e-3, gate is 2e-2).

Mask handling (same algebra as v1, but folded into host-side data):
  xq rows 0..77 hold x^T pre-multiplied by m_i, row 79 = m_i, so the q
  projection emits m_i*q and a m_i column with no on-chip mask multiply.
  xkv rows 0..77 = x^T, row 78 = m_j, row 79 = 1; the k weights put
  +-30 in the paired column so dots'^T[j,i] = m_i*qk + m_i*(30 m_j - 30).
  Fully-masked rows become exp(0)=1 -> uniform 1/512 (reference semantics);
  masked keys get exp(-30)~9e-14. Rowsums ride ones-columns in V.
  adj term: (0.5*adj)@(m_j*v) with 0.5 folded into the host adj transpose.

Pipeline: per token-tile jt the dots matmuls fill three 2-head psum
chunks; chunk ci owns dots pool ci (2 banks each; ACT exps chunks on
pools 0/1, DVE fast-exps on pool 2) so each engine's next chunk is
always pre-fillable and the exp streams run gap-free. Attention@V plus
adj@vm accumulate progressively into two it-pair psum banks as each
jt's exps land; softmax division reads rowsums (ones-columns in V)
straight from psum. Projections of batch b+1 are threaded through the
dots pools during batch b's exps. Engine assignment of every drain is
tuned via DRAIN_SCHED / EXP_SCHED against the TimelineSim trace.
Output: combine, PE transpose, out-proj, bf16 DMA out (host upcasts).
"""

import os
import numpy as np
import ml_dtypes

H, DH, DIM = 6, 13, 78
LA, LG = 1.0, 0.5
B, N = 16, 512
SCALE = DH ** -0.5
NEG = 30.0
NCORES = 8
BPC = B // NCORES          # batches per core
NT = N // 128              # 128-token tiles per sequence
HDA = 14                   # dh + 1 (ones column) per head in V_aug
HPP = 3                    # heads per pass (A: 0-2, B: 3-5)
LOG2E = 1.4426950408889634
FE_A = 128.0 * LOG2E       # fast-exp scale
FE_B = 127.0 * 128.0 - 128.0 * 0.043   # fast-exp bias (tuned c)
ATTN_W = 79                # attn cols per it (78 + ones col for bias)
PE_WARMUP = 10             # identity transposes to ramp the PE p-state

# exp engine schedule: (b, jt, ci) -> "act" | "dve", where chunk ci covers
# heads (2ci, 2ci+1). ACT is cheaper per element (0.833 vs 1.042 ns) so it
# gets the majority; DVE runs the fast-exp on its share.
EXP_SCHED = {}
for _b in range(BPC):
    for _jt in range(NT):
        for _ci in range(3):
            EXP_SCHED[(_b, _jt, _ci)] = "dve" if _ci == 2 else "act"
# trace-tuned extras: shift exp work toward DVE where its queue has
# slack; swap the last jt's DVE chunk to ci1 so the final-tail skew
# between the two exp streams is minimal
EXP_SCHED[(1, 0, 1)] = "dve"
EXP_SCHED[(0, 1, 1)] = "dve"
EXP_SCHED[(1, 3, 2)] = "act"
EXP_SCHED[(1, 3, 1)] = "dve"

# engine for each drain / post op, per batch: tuned against the trace.
DRAIN_SCHED = {
    ("q0", 0): "act", ("k0", 0): "dve", ("q1", 0): "dve", ("k1", 0): "dve",
    ("va", 0): "dve",
    ("q0", 1): "act", ("k0", 1): "act", ("q1", 1): "dve", ("k1", 1): "act",
    ("va", 1): "dve",
    ("pv0", 0): "dve", ("pv1", 0): "dve",
    ("pv0", 1): "act", ("pv1", 1): "act",
    ("outT", 0): "dve", ("outT", 1): "act",
    ("ysb", 0): "dve", ("ysb", 1): "dve",
}

_CACHE = {}


# ---------------------------------------------------------------------------
# Workaround: this container's walrus rejects the multi-wait Drain that
# TileContext emits at exit ("Too many sync wait commands"). Split the waits
# into individual wait_ge instructions on the SP engine before a bare drain.
def _apply_tile_patch(tile_mod, ScopedClock):
    def _patched(self, tick_clock, wait_clock):
        nc = self.nc
        drain_inst = nc.sync.drain()
        wait_clock.add_sem_waits(
            drain_inst.ins, ScopedClock({None: tick_clock.global_clock})
        )
        mi = drain_inst.ins
        waits = list(mi.sync_info.on_wait)
        if len(waits) > 1:
            handles = {s.name: s for s in self.sems.allocated().values()}
            engines = [nc.sync, nc.vector, nc.scalar, nc.tensor, nc.gpsimd]
            kept = []
            k = 0
            for w in waits:
                h = handles.get(w.ant_name)
                if h is None:
                    kept.append(w)
                    continue
                engines[k % len(engines)].wait_ge(h, w.wait_value)
                k += 1
            mi.sync_info.on_wait = kept
        nc.all_engine_barrier()
        assert self.sems is not None
        popped = nc._tile_sem_poison_stack.pop()
        assert popped is self._sem_poison
        # no barrier after the sem clears: they ride the SP stream, which
        # the runtime waits out anyway before the NEFF completes
        nc.clear_and_free_semaphores(list(self.sems.allocated().values()))

    tile_mod.TileContext._drain_and_barrier = _patched


def _split_waits(nc, mybir):
    """This walrus build only encodes one sem-wait per instruction; hoist
    extra waits onto same-engine NoOps inserted right before the owner."""
    k = 0
    for f in nc.m.functions:
        for bb in f.blocks:
            out = []
            changed = False
            for inst in bb.instructions:
                si = inst.sync_info
                waits = list(si.on_wait) if si is not None else []
                if len(waits) > 1:
                    changed = True
                    for w in waits[:-1]:
                        n = mybir.InstNoOp(name=f"I-wsplit-{k}", ins=[], outs=[])
                        k += 1
                        n.engine = inst.engine
                        n.sync_info = mybir.SyncInfo(on_wait=[w], on_update=[])
                        out.append(n)
                    si.on_wait = [waits[-1]]
                out.append(inst)
            if changed:
                bb.instructions = out


# ---------------------------------------------------------------------------
def _host_weights(W_qkv, W_out, b_out):
    """Stationary weights, bf16: [wqA wqB wkA wkB | wv | wo]."""
    W = W_qkv.reshape(DIM, H, 3, DH).astype(np.float32)

    def qk_stack(heads, kind):
        w = np.zeros((80, 128), np.float32)
        for g, h in enumerate(heads):
            c0 = 32 * g
            if kind == "q":
                w[0:DIM, c0:c0 + DH] = W[:, h, 0, :] * SCALE
                w[79, c0 + DH] = 1.0          # xq row 79 = m_i -> col is m_i
            else:
                w[0:DIM, c0:c0 + DH] = W[:, h, 1, :]
                w[78, c0 + DH] = NEG          # xkv row 78 = m_j -> +30*m_j
                w[79, c0 + DH] = -NEG         # xkv row 79 = 1   -> -30
        return w

    pa, pb = [0, 1, 2], [3, 4, 5]
    wqa, wqb = qk_stack(pa, "q"), qk_stack(pb, "q")
    wka, wkb = qk_stack(pa, "k"), qk_stack(pb, "k")

    wv = np.zeros((80, H * HDA), np.float32)
    for h in range(H):
        wv[0:DIM, h * HDA:h * HDA + DH] = W[:, h, 2, :]

    wo = np.zeros((80, DIM), np.float32)
    wo[0:DIM, :] = W_out.astype(np.float32)
    wo[78, :] = b_out.astype(np.float32)      # attn ones col -> bias
    full = np.concatenate([wqa, wqb, wka, wkb, wv, wo], axis=1)
    return full.astype(ml_dtypes.bfloat16)


WCOLS = 4 * 128 + H * HDA + DIM


def _build_bass(walrus_patches=True):
    import concourse.bass as bass
    import concourse.mybir as mybir
    import concourse.tile as tile
    from concourse.vector_clock import ScopedClock
    from concourse.masks import make_identity

    if walrus_patches:
        _apply_tile_patch(tile, ScopedClock)

    f32 = mybir.dt.float32
    bf16 = mybir.dt.bfloat16
    i16 = mybir.dt.int16
    AF = mybir.ActivationFunctionType
    OP = mybir.AluOpType

    nc = bass.Bass()
    # xw0: [xq | xkv | weights] for batch 0 in ONE DMA (startup critical
    # path pays one HWDGE setup + one DMA-sem hop instead of two);
    # xall1: [xq | xkv] for batch 1; adjm: adj tiles + mask col per tile
    xw0_d = nc.dram_tensor("xw0", [80, 2 * N + WCOLS], bf16,
                           kind="ExternalInput")
    xall1_d = nc.dram_tensor("xall1", [80, 2 * N], bf16, kind="ExternalInput")
    adjm_d = nc.dram_tensor("adjm", [BPC, 128, NT * (N + 1)], bf16,
                            kind="ExternalInput")
    yout = nc.dram_tensor("yout", [BPC, N, DIM], bf16, kind="ExternalOutput")

    with tile.TileContext(nc) as tc:
        with (
            tc.tile_pool(name="consts", bufs=1) as consts,
            tc.tile_pool(name="bpool", bufs=2) as bpool,
            tc.tile_pool(name="ptp", bufs=2 * NT) as ptp,
            tc.tile_pool(name="spool", bufs=4) as spool,
            tc.tile_pool(name="opool", bufs=2) as opool,
            tc.tile_pool(name="ps_d0", bufs=1, space="PSUM") as ps_d0,
            tc.tile_pool(name="ps_d1", bufs=1, space="PSUM") as ps_d1,
            tc.tile_pool(name="ps_d2", bufs=1, space="PSUM") as ps_d2,
            tc.tile_pool(name="ps_av0", bufs=1, space="PSUM") as ps_av0,
            tc.tile_pool(name="ps_av1", bufs=1, space="PSUM") as ps_av1,
        ):
            PSD = [ps_d0, ps_d1, ps_d2]
            PSA = [ps_av0, ps_av1]
            identity = consts.tile([128, 128], bf16)
            make_identity(nc, identity)
            # warm-up exp hoists the one-time ACT table load off the
            # first real exp's critical path
            warm = consts.tile([128, 1], f32, tag="warm")
            nc.vector.memset(warm, 0.0)
            nc.scalar.activation(warm[:], warm[:], AF.Exp)
            xw0 = consts.tile([80, 2 * N + WCOLS], bf16, tag="xw0")
            nc.sync.dma_start(xw0[:], xw0_d[:])
            wall = xw0[:, 2 * N:]
            wq = [wall[:, 0:128], wall[:, 128:256]]
            wk = [wall[:, 256:384], wall[:, 384:512]]
            wv = wall[:, 512:512 + H * HDA]
            wo = wall[0:79, 512 + H * HDA:WCOLS]
            # PE p-state warm-up: keep the tensor engine continuously busy
            # from t~0 so real matmuls run at the ramped rate. P2 is the
            # last pool needed by real work.
            for wi in range(PE_WARMUP):
                pw = ps_d2.tile([128, 128], bf16, tag="psd2")
                nc.tensor.transpose(pw[:], identity[:], identity[:])

            DR = {"act": nc.scalar.copy, "dve": nc.vector.tensor_copy}

            def dma_in(b):
                if b == 0:
                    xall = xw0[:, 0:2 * N]
                else:
                    xall1 = bpool.tile([80, 2 * N], bf16, tag="xall")
                    nc.sync.dma_start(xall1[:], xall1_d[:])
                    xall = xall1[:]
                adjm = bpool.tile([128, NT, N + 1], bf16, tag="adjm")
                nc.sync.dma_start(adjm[:], adjm_d[b].rearrange(
                    "p (t i) -> p t i", t=NT))
                mcol = spool.tile([128, NT], f32, tag="mcol")
                nc.vector.tensor_copy(mcol[:], adjm[:, :, N])
                return dict(xq=xall[:, 0:N], xkv=xall[:, N:2 * N],
                            adjs=adjm[:, :, 0:N], mcol=mcol,
                            qs=[None, None], ks=[None, None])

            def proj_qk(b, st, which, pool):
                """One q or k projection ('q0','k0','q1','k1') via `pool`."""
                kind, s = which[0], int(which[1])
                w = (wq if kind == "q" else wk)[s]
                x = st["xq"] if kind == "q" else st["xkv"]
                ps = pool.tile([128, N], f32, tag=pool.name.replace("ps_", "ps"))
                nc.tensor.matmul(ps[:], w, x)
                sb = bpool.tile([128, N], bf16, tag=which)
                DR[DRAIN_SCHED[(which, b)]](sb[:], ps[:])
                (st["qs"] if kind == "q" else st["ks"])[s] = sb

            def proj_v(b, st, pool):
                psv = pool.tile([128, NT * H * HDA], f32, tag=pool.name.replace("ps_", "ps"))
                for t in range(NT):
                    nc.tensor.matmul(
                        psv[:, t * H * HDA:(t + 1) * H * HDA],
                        st["xkv"][0:DIM, t * 128:(t + 1) * 128], wv[0:DIM, :],
                        start=(t == 0), stop=(t == NT - 1),
                        skip_group_check=True)
                va = bpool.tile([128, NT, H * HDA], bf16, tag="va")
                DR[DRAIN_SCHED[("va", b)]](
                    va[:].rearrange("p t c -> p (t c)"), psv[:])
                vm = bpool.tile([128, NT, H * DH], bf16, tag="vm")
                for t in range(NT):
                    nc.gpsimd.tensor_tensor(
                        vm[:, t, :].rearrange("p (h c) -> p h c", c=DH),
                        va[:, t, :].rearrange("p (h c) -> p h c", c=HDA)[:, :, 0:DH],
                        st["mcol"][:, t:t + 1].unsqueeze(2).broadcast_to(
                            [128, H, DH]),
                        op=OP.mult)
                nc.gpsimd.memset(
                    va[:].rearrange("p t (h c) -> p t h c", c=HDA)[:, :, :, DH:HDA],
                    1.0)
                st["va"], st["vm"] = va, vm

            def emit_dots(b, jt, st, cis=(0, 1, 2), pts=None):
                """Three 2-head chunks for token tile jt; chunk ci owns
                pool ci (ACT: P0/P1, DVE: P2) so each engine's next chunk
                is always pre-fillable."""
                jsl = slice(jt * 128, (jt + 1) * 128)
                if pts is None:
                    pts = []
                for ci in cis:
                    psd = PSD[ci].tile([128, 2, N], f32, tag=f"psd{ci}")
                    for k in range(2):
                        h = 2 * ci + k
                        t, g = divmod(h, HPP)
                        c = 32 * g
                        nc.tensor.matmul(
                            psd[:, k, :], st["ks"][t][c:c + DH + 1, jsl],
                            st["qs"][t][c:c + DH + 1, :],
                            tile_position=(c, 0))
                    eng = EXP_SCHED[(b, jt, ci)]
                    if eng == "act":
                        pt = ptp.tile([128, 2, N], bf16, tag=f"pt{ci}")
                        nc.scalar.activation(
                            pt[:].rearrange("p h n -> p (h n)"),
                            psd[:].rearrange("p h n -> p (h n)"), AF.Exp)
                    else:
                        pt = ptp.tile([128, 2, N], i16, tag=f"pt{ci}")
                        nc.vector.tensor_scalar(
                            pt[:].rearrange("p h n -> p (h n)"),
                            psd[:].rearrange("p h n -> p (h n)"),
                            FE_A, FE_B, op0=OP.mult, op1=OP.add)
                    pts.append(pt)
                return pts

            def emit_av_jt(b, st, psos, pts_jt, jt, parts=("adj", 0, 1, 2)):
                """Accumulate this jt's attention@V / adj@vm contributions
                into the two it-pair psum banks. `parts` selects which
                chunk's head-matmuls (0/1/2) or the adjacency ("adj") are
                emitted, so each piece only queues behind the exp it needs.
                The psum group start is the first jt0 adj matmul and the
                stop is the last jt3 ci2 matmul (same-engine in-order
                execution keeps the zeroing first and the stop last)."""
                for part in parts:
                    for itp in range(2):
                        pso = psos[itp]
                        for ki in range(2):
                            it = 2 * itp + ki
                            isl = slice(it * 128, (it + 1) * 128)
                            if part == "adj":
                                nc.tensor.matmul(
                                    pso[:, ki, H * HDA:],
                                    st["adjs"][:, jt, isl],
                                    st["vm"][:, jt, :],
                                    start=(jt == 0 and ki == 0),
                                    stop=False, skip_group_check=True)
                                continue
                            for hh in range(2):
                                h = 2 * part + hh
                                pt = pts_jt[part]
                                ptv = pt[:, hh, isl]
                                if pt.dtype != bf16:
                                    ptv = ptv.bitcast(bf16)
                                nc.tensor.matmul(
                                    pso[:, ki, h * HDA:(h + 1) * HDA], ptv,
                                    st["va"][:, jt, h * HDA:(h + 1) * HDA],
                                    start=False,
                                    stop=(jt == NT - 1 and part == 2
                                          and ki == 1 and hh == 1),
                                    skip_group_check=True)

            def emit_post(b, st, psos, attn):
                """drain AV psum; softmax divide (Pool) + adj add -> attn."""
                for itp in range(2):
                    pso = psos[itp]
                    pv = opool.tile([128, 2, H * HDA + H * DH], bf16,
                                    tag=f"pv{itp}")
                    DR[DRAIN_SCHED[(f"pv{itp}", b)]](
                        pv[:].rearrange("p k c -> p (k c)"),
                        pso[:].rearrange("p k c -> p (k c)"))
                    pvh = pv[:, :, 0:H * HDA].rearrange(
                        "p k (h c) -> p k h c", c=HDA)
                    rs = spool.tile([128, 2, H], f32, tag=f"rs{itp}")
                    nc.vector.reciprocal(
                        rs[:].unsqueeze(3), pvh[:, :, :, DH:HDA])
                    t1 = spool.tile([128, 2, H * DH], bf16, tag=f"t1{itp}")
                    eng = (nc.gpsimd if itp == 0 else nc.vector)
                    eng.tensor_tensor(
                        t1[:].rearrange("p k (h c) -> p k h c", c=DH),
                        pvh[:, :, :, 0:DH],
                        rs[:].unsqueeze(3).broadcast_to([128, 2, H, DH]),
                        op=OP.mult)
                    for ki in range(2):
                        it = 2 * itp + ki
                        nc.vector.scalar_tensor_tensor(
                            attn[:, it, 0:H * DH],
                            pv[:, ki, H * HDA:], st["mcol"][:, it:it + 1],
                            t1[:, ki, :], op0=OP.mult, op1=OP.add)

            def emit_out(b, st, attn):
                """transpose + output projection + store, all 4 its."""
                outT = bpool.tile([79, N], bf16, tag="outT")
                ysb = opool.tile([128, NT, DIM], bf16, tag="ysb")
                # psy lives in PSD1; psa ping-pongs PSD0/PSD2 (no shared
                # pool, so interleaving transposes and psy matmuls is safe)
                psy = PSD[1].tile([128, NT * DIM], f32, tag="psd1")
                for it in range(NT):
                    isl = slice(it * 128, (it + 1) * 128)
                    pool = PSD[2 * (it % 2)]
                    psa = pool.tile([79, 128], bf16,
                                    tag=pool.name.replace("ps_", "ps"))
                    nc.tensor.transpose(
                        psa[:], attn[:, it, 0:79], identity[:])
                    DR[DRAIN_SCHED[("outT", b)]](outT[:, isl], psa[:])
                    # two psum groups (its 0-1 / 2-3) in one bank: the
                    # second group's start=False rides the zeroing done by
                    # the first, so each half can be drained and DMA'd as
                    # soon as its own stop lands (overlaps the final DMA
                    # setup with the second half's compute)
                    nc.tensor.matmul(
                        psy[:, it * DIM:(it + 1) * DIM],
                        outT[:, isl], wo[:],
                        start=(it == 0), stop=(it in (1, NT - 1)),
                        skip_group_check=True)
                    if b == BPC - 1 and it in (1, NT - 1):
                        half = slice(0, 2) if it == 1 else slice(2, NT)
                        DR[DRAIN_SCHED[("ysb", b)]](
                            ysb[:, half, :].rearrange("p t f -> p (t f)"),
                            psy[:, half.start * DIM:half.stop * DIM])
                        nc.sync.dma_start(
                            yout[b].rearrange("(t p) f -> p t f", p=128)
                            [:, half, :], ysb[:, half, :])
                if b != BPC - 1:
                    DR[DRAIN_SCHED[("ysb", b)]](
                        ysb[:].rearrange("p t f -> p (t f)"), psy[:])
                    nc.sync.dma_start(
                        yout[b].rearrange("(t p) f -> p t f", p=128), ysb[:])

            def new_attn():
                attn = bpool.tile([128, NT, ATTN_W], bf16, tag="attn")
                nc.gpsimd.memset(attn[:, :, DIM:ATTN_W], 1.0)
                return attn

            def new_psos():
                return [PSA[itp].tile([128, 2, H * HDA + H * DH], f32,
                                      tag=f"av{itp}", name=f"pso{itp}")
                        for itp in range(2)]

            # ---- emission schedule (per-engine queues execute in this
            # order; chosen so the exp streams never sit behind later-dep
            # work in their queues) ----
            st0 = dma_in(0)
            proj_qk(0, st0, "q0", PSD[0])
            proj_qk(0, st0, "k0", PSD[1])
            proj_qk(0, st0, "q1", PSD[2])
            psos0 = new_psos()
            attn0 = new_attn()
            p00 = emit_dots(0, 0, st0, cis=(0,))
            proj_qk(0, st0, "k1", PSD[1])
            emit_dots(0, 0, st0, cis=(1,), pts=p00)
            proj_v(0, st0, PSD[2])
            emit_dots(0, 0, st0, cis=(2,), pts=p00)
            p01 = emit_dots(0, 1, st0)
            emit_av_jt(0, st0, psos0, p00, 0, parts=("adj", 0, 1))
            st1 = dma_in(1)
            proj_qk(1, st1, "q0", PSD[0])
            proj_qk(1, st1, "k0", PSD[1])
            proj_qk(1, st1, "q1", PSD[2])
            p02 = emit_dots(0, 2, st0)
            emit_av_jt(0, st0, psos0, p00, 0, parts=(2,))
            emit_av_jt(0, st0, psos0, p01, 1, parts=("adj", 0, 1))
            proj_qk(1, st1, "k1", PSD[0])
            proj_v(1, st1, PSD[1])
            p03 = emit_dots(0, 3, st0)
            emit_av_jt(0, st0, psos0, p01, 1, parts=(2,))
            emit_av_jt(0, st0, psos0, p02, 2, parts=("adj", 0, 1))
            pts1 = [emit_dots(1, 0, st1)]
            emit_av_jt(0, st0, psos0, p02, 2, parts=(2,))
            emit_av_jt(0, st0, psos0, p03, 3, parts=("adj", 0, 1))
            pts1.append(emit_dots(1, 1, st1))
            emit_av_jt(0, st0, psos0, p03, 3, parts=(2,))
            emit_post(0, st0, psos0, attn0)
            pts1.append(emit_dots(1, 2, st1))
            psos1 = new_psos()
            attn1 = new_attn()
            emit_av_jt(1, st1, psos1, pts1[0], 0)
            emit_av_jt(1, st1, psos1, pts1[1], 1, parts=("adj", 0, 1))
            pts1.append(emit_dots(1, 3, st1))
            emit_av_jt(1, st1, psos1, pts1[1], 1, parts=(2,))
            emit_av_jt(1, st1, psos1, pts1[2], 2)
            emit_out(0, st0, attn0)
            emit_av_jt(1, st1, psos1, pts1[3], 3)
            emit_post(1, st1, psos1, attn1)
            emit_out(1, st1, attn1)

    if walrus_patches:
        _split_waits(nc, mybir)
    return nc


def _prep_inputs(x, mask, adjacency_mat, W_qkv, W_out, b_out):
    x = np.asarray(x, np.float32)
    maskf = np.ascontiguousarray(np.asarray(mask, np.float32))
    adj = np.asarray(adjacency_mat, np.float32)
    wall = _host_weights(
        np.asarray(W_qkv, np.float32), np.asarray(W_out, np.float32),
        np.asarray(b_out, np.float32))
    xt = x.transpose(0, 2, 1)                      # [B, DIM, N]
    # xall: [xq | xkv] side by side (one DMA per batch)
    xall = np.zeros((B, 80, 2 * N), np.float32)
    xall[:, 0:DIM, 0:N] = xt * maskf[:, None, :]   # pre-masked x^T
    xall[:, 79, 0:N] = maskf
    xall[:, 0:DIM, N:2 * N] = xt
    xall[:, 78, N:2 * N] = maskf
    xall[:, 79, N:2 * N] = 1.0
    xall = xall.astype(ml_dtypes.bfloat16)
    wallf = np.asarray(wall)
    # adjm: per 128-row tile, [adj^T * 0.5 tile cols | mask col]
    adjt = (adj * LG).transpose(0, 2, 1)           # [B, j, i]
    adjm = np.zeros((B, 128, NT, N + 1), np.float32)
    adjm[:, :, :, 0:N] = adjt.reshape(B, NT, 128, N).transpose(0, 2, 1, 3)
    adjm[:, :, :, N] = maskf.reshape(B, NT, 128).transpose(0, 2, 1)
    adjm = adjm.reshape(B, 128, NT * (N + 1)).astype(ml_dtypes.bfloat16)
    in_maps = []
    for c in range(NCORES):
        s = slice(c * BPC, (c + 1) * BPC)
        b0 = c * BPC
        in_maps.append({
            "xw0": np.ascontiguousarray(
                np.concatenate([xall[b0], wallf], axis=1)),
            "xall1": np.ascontiguousarray(xall[b0 + 1]),
            "adjm": np.ascontiguousarray(adjm[s]),
        })
    return in_maps


LAST_EXEC_NS = None
LAST_RESULT = None


def kernel(x, mask, adjacency_mat, W_qkv, W_out, b_out):
    global LAST_EXEC_NS, LAST_RESULT
    from concourse.bass_utils import run_bass_kernel_spmd

    if "nc" not in _CACHE:
        _CACHE["nc"] = _build_bass()
    nc = _CACHE["nc"]

    in_maps = _prep_inputs(x, mask, adjacency_mat, W_qkv, W_out, b_out)
    trace = bool(int(os.environ.get("KERNEL_TRACE", "0")))
    res = run_bass_kernel_spmd(
        nc, in_maps, core_ids=list(range(NCORES)), trace=trace)
    LAST_EXEC_NS = res.exec_time_ns
    LAST_RESULT = res
    y = np.concatenate(
        [np.asarray(res.results[c]["yout"]).astype(np.float32)
         for c in range(NCORES)], axis=0)
    return np.ascontiguousarray(y)


# revision 48
# speedup vs baseline: 1.2687x; 1.0152x over previous
"""Trainium2 Bass kernel for nn_Attention_5145370821223.

Computation (per batch b of 16, heads H=6, tokens N=512, dim 78, dh 13):
    qkv = x @ W_qkv ; dots = q k^T / sqrt(13), masked by m_i & m_j
    attn = softmax(dots) * 1.0 + 0.5 * adj * (m_i & m_j)
    y = (attn @ v) @ W_out + b_out

Strategy: data-parallel over batch (2 batches per NeuronCore x 8 cores).
The softmax exponentials dominate (6*512*512 elems/batch); they are split
across BOTH the scalar engine (exact Exp activation) and the vector engine
(one-instruction Schraudolph fast-exp: int16(128*log2e*x + 16250.496)
bitcast to bf16, max ~3% elementwise rel err which cancels between softmax
numerator and denominator; end-to-end rel err ~2.5e-3, gate is 2e-2).

Mask handling (same algebra as v1, but folded into host-side data):
  xq rows 0..77 hold x^T pre-multiplied by m_i, row 79 = m_i, so the q
  projection emits m_i*q and a m_i column with no on-chip mask multiply.
  xkv rows 0..77 = x^T, row 78 = m_j, row 79 = 1; the k weights put
  +-30 in the paired column so dots'^T[j,i] = m_i*qk + m_i*(30 m_j - 30).
  Fully-masked rows become exp(0)=1 -> uniform 1/512 (reference semantics);
  masked keys get exp(-30)~9e-14. Rowsums ride ones-columns in V.
  adj term: (0.5*adj)@(m_j*v) with 0.5 folded into the host adj transpose.

Pipeline: per token-tile jt the dots matmuls fill three 2-head psum
chunks; chunk ci owns dots pool ci (2 banks each; ACT exps chunks on
pools 0/1, DVE fast-exps on pool 2) so each engine's next chunk is
always pre-fillable and the exp streams run gap-free. Attention@V plus
adj@vm accumulate progressively into two it-pair psum banks as each
jt's exps land; softmax division reads rowsums (ones-columns in V)
straight from psum. Projections of batch b+1 are threaded through the
dots pools during batch b's exps. Engine assignment of every drain is
tuned via DRAIN_SCHED / EXP_SCHED against the TimelineSim trace.
Output: combine, PE transpose, out-proj, bf16 DMA out (host upcasts).
"""

import os
import numpy as np
import ml_dtypes

H, DH, DIM = 6, 13, 78
LA, LG = 1.0, 0.5
B, N = 16, 512
SCALE = DH ** -0.5
NEG = 30.0
NCORES = 8
BPC = B // NCORES          # batches per core
NT = N // 128              # 128-token tiles per sequence
HDA = 14                   # dh + 1 (ones column) per head in V_aug
HPP = 3                    # heads per pass (A: 0-2, B: 3-5)
LOG2E = 1.4426950408889634
FE_A = 128.0 * LOG2E       # fast-exp scale
FE_B = 127.0 * 128.0 - 128.0 * 0.043   # fast-exp bias (tuned c)
ATTN_W = 79                # attn cols per it (78 + ones col for bias)
PE_WARMUP = 10             # identity transposes to ramp the PE p-state

# exp engine schedule: (b, jt, ci) -> "act" | "dve", where chunk ci covers
# heads (2ci, 2ci+1). ACT is cheaper per element (0.833 vs 1.042 ns) so it
# gets the majority; DVE runs the fast-exp on its share.
EXP_SCHED = {}
for _b in range(BPC):
    for _jt in range(NT):
        for _ci in range(3):
            EXP_SCHED[(_b, _jt, _ci)] = "dve" if _ci == 2 else "act"
# trace-tuned extras: shift exp work toward DVE where its queue has
# slack; swap the last jt's DVE chunk to ci1 so the final-tail skew
# between the two exp streams is minimal
EXP_SCHED[(1, 0, 1)] = "dve"
EXP_SCHED[(0, 1, 1)] = "dve"
EXP_SCHED[(1, 3, 2)] = "act"
EXP_SCHED[(1, 3, 1)] = "dve"

# engine for each drain / post op, per batch: tuned against the trace.
DRAIN_SCHED = {
    ("q0", 0): "act", ("k0", 0): "dve", ("q1", 0): "dve", ("k1", 0): "dve",
    ("va", 0): "dve",
    ("q0", 1): "act", ("k0", 1): "act", ("q1", 1): "dve", ("k1", 1): "act",
    ("va", 1): "dve",
    ("pv0", 0): "dve", ("pv1", 0): "dve",
    ("pv0", 1): "act", ("pv1", 1): "act",
    ("outT", 0): "dve", ("outT", 1): "act",
    ("ysb", 0): "dve", ("ysb", 1): "dve",
}

_CACHE = {}


# ---------------------------------------------------------------------------
# Workaround: this container's walrus rejects the multi-wait Drain that
# TileContext emits at exit ("Too many sync wait commands"). Split the waits
# into individual wait_ge instructions on the SP engine before a bare drain.
def _apply_tile_patch(tile_mod, ScopedClock):
    def _patched(self, tick_clock, wait_clock):
        nc = self.nc
        drain_inst = nc.sync.drain()
        wait_clock.add_sem_waits(
            drain_inst.ins, ScopedClock({None: tick_clock.global_clock})
        )
        mi = drain_inst.ins
        waits = list(mi.sync_info.on_wait)
        if len(waits) > 1:
            handles = {s.name: s for s in self.sems.allocated().values()}
            engines = [nc.sync, nc.vector, nc.scalar, nc.tensor, nc.gpsimd]
            kept = []
            k = 0
            for w in waits:
                h = handles.get(w.ant_name)
                if h is None:
                    kept.append(w)
                    continue
                engines[k % len(engines)].wait_ge(h, w.wait_value)
                k += 1
            mi.sync_info.on_wait = kept
        nc.all_engine_barrier()
        assert self.sems is not None
        popped = nc._tile_sem_poison_stack.pop()
        assert popped is self._sem_poison
        # no barrier after the sem clears: they ride the SP stream, which
        # the runtime waits out anyway before the NEFF completes
        nc.clear_and_free_semaphores(list(self.sems.allocated().values()))

    tile_mod.TileContext._drain_and_barrier = _patched


def _split_waits(nc, mybir):
    """This walrus build only encodes one sem-wait per instruction; hoist
    extra waits onto same-engine NoOps inserted right before the owner."""
    k = 0
    for f in nc.m.functions:
        for bb in f.blocks:
            out = []
            changed = False
            for inst in bb.instructions:
                si = inst.sync_info
                waits = list(si.on_wait) if si is not None else []
                if len(waits) > 1:
                    changed = True
                    for w in waits[:-1]:
                        n = mybir.InstNoOp(name=f"I-wsplit-{k}", ins=[], outs=[])
                        k += 1
                        n.engine = inst.engine
                        n.sync_info = mybir.SyncInfo(on_wait=[w], on_update=[])
                        out.append(n)
                    si.on_wait = [waits[-1]]
                out.append(inst)
            if changed:
                bb.instructions = out


# ---------------------------------------------------------------------------
def _host_weights(W_qkv, W_out, b_out):
    """Stationary weights, bf16: [wqA wqB wkA wkB | wv | wo]."""
    W = W_qkv.reshape(DIM, H, 3, DH).astype(np.float32)

    def qk_stack(heads, kind):
        w = np.zeros((80, 128), np.float32)
        for g, h in enumerate(heads):
            c0 = 32 * g
            if kind == "q":
                w[0:DIM, c0:c0 + DH] = W[:, h, 0, :] * SCALE
                w[79, c0 + DH] = 1.0          # xq row 79 = m_i -> col is m_i
            else:
                w[0:DIM, c0:c0 + DH] = W[:, h, 1, :]
                w[78, c0 + DH] = NEG          # xkv row 78 = m_j -> +30*m_j
                w[79, c0 + DH] = -NEG         # xkv row 79 = 1   -> -30
        return w

    pa, pb = [0, 1, 2], [3, 4, 5]
    wqa, wqb = qk_stack(pa, "q"), qk_stack(pb, "q")
    wka, wkb = qk_stack(pa, "k"), qk_stack(pb, "k")

    wv = np.zeros((80, H * HDA), np.float32)
    for h in range(H):
        wv[0:DIM, h * HDA:h * HDA + DH] = W[:, h, 2, :]

    wo = np.zeros((80, DIM), np.float32)
    wo[0:DIM, :] = W_out.astype(np.float32)
    wo[78, :] = b_out.astype(np.float32)      # attn ones col -> bias
    full = np.concatenate([wqa, wqb, wka, wkb, wv, wo], axis=1)
    return full.astype(ml_dtypes.bfloat16)


WCOLS = 4 * 128 + H * HDA + DIM


def _build_bass(walrus_patches=True):
    import concourse.bass as bass
    import concourse.mybir as mybir
    import concourse.tile as tile
    from concourse.vector_clock import ScopedClock
    from concourse.masks import make_identity

    if walrus_patches:
        _apply_tile_patch(tile, ScopedClock)

    f32 = mybir.dt.float32
    bf16 = mybir.dt.bfloat16
    i16 = mybir.dt.int16
    AF = mybir.ActivationFunctionType
    OP = mybir.AluOpType

    nc = bass.Bass()
    # xw0: [xq | xkv | weights] for batch 0 in ONE DMA (startup critical
    # path pays one HWDGE setup + one DMA-sem hop instead of two);
    # xall1: [xq | xkv] for batch 1; adjm: adj tiles + mask col per tile
    xw0_d = nc.dram_tensor("xw0", [80, 2 * N + WCOLS], bf16,
                           kind="ExternalInput")
    xall1_d = nc.dram_tensor("xall1", [80, 2 * N], bf16, kind="ExternalInput")
    adjm_d = nc.dram_tensor("adjm", [BPC, 128, NT * (N + 1)], bf16,
                            kind="ExternalInput")
    yout = nc.dram_tensor("yout", [BPC, N, DIM], bf16, kind="ExternalOutput")

    with tile.TileContext(nc) as tc:
        with (
            tc.tile_pool(name="consts", bufs=1) as consts,
            tc.tile_pool(name="bpool", bufs=2) as bpool,
            tc.tile_pool(name="ptp", bufs=2 * NT) as ptp,
            tc.tile_pool(name="spool", bufs=4) as spool,
            tc.tile_pool(name="opool", bufs=2) as opool,
            tc.tile_pool(name="ps_d0", bufs=1, space="PSUM") as ps_d0,
            tc.tile_pool(name="ps_d1", bufs=1, space="PSUM") as ps_d1,
            tc.tile_pool(name="ps_d2", bufs=1, space="PSUM") as ps_d2,
            tc.tile_pool(name="ps_av0", bufs=1, space="PSUM") as ps_av0,
            tc.tile_pool(name="ps_av1", bufs=1, space="PSUM") as ps_av1,
        ):
            PSD = [ps_d0, ps_d1, ps_d2]
            PSA = [ps_av0, ps_av1]
            identity = consts.tile([128, 128], bf16)
            make_identity(nc, identity)
            # warm-up exp hoists the one-time ACT table load off the
            # first real exp's critical path
            warm = consts.tile([128, 1], f32, tag="warm")
            nc.vector.memset(warm, 0.0)
            nc.scalar.activation(warm[:], warm[:], AF.Exp)
            xw0 = consts.tile([80, 2 * N + WCOLS], bf16, tag="xw0")
            nc.sync.dma_start(xw0[:], xw0_d[:])
            wall = xw0[:, 2 * N:]
            wq = [wall[:, 0:128], wall[:, 128:256]]
            wk = [wall[:, 256:384], wall[:, 384:512]]
            wv = wall[:, 512:512 + H * HDA]
            wo = wall[0:79, 512 + H * HDA:WCOLS]
            # PE p-state warm-up: keep the tensor engine continuously busy
            # from t~0 so real matmuls run at the ramped rate. P2 is the
            # last pool needed by real work.
            for wi in range(PE_WARMUP):
                pw = ps_d2.tile([128, 128], bf16, tag="psd2")
                nc.tensor.transpose(pw[:], identity[:], identity[:])

            DR = {"act": nc.scalar.copy, "dve": nc.vector.tensor_copy}

            def dma_in(b):
                if b == 0:
                    xall = xw0[:, 0:2 * N]
                else:
                    xall1 = bpool.tile([80, 2 * N], bf16, tag="xall")
                    nc.sync.dma_start(xall1[:], xall1_d[:])
                    xall = xall1[:]
                adjm = bpool.tile([128, NT, N + 1], bf16, tag="adjm")
                nc.sync.dma_start(adjm[:], adjm_d[b].rearrange(
                    "p (t i) -> p t i", t=NT))
                return dict(xq=xall[:, 0:N], xkv=xall[:, N:2 * N],
                            adjs=adjm[:, :, 0:N], adjm=adjm,
                            qs=[None, None], ks=[None, None])

            def proj_qk(b, st, which, pool):
                """One q or k projection ('q0','k0','q1','k1') via `pool`."""
                kind, s = which[0], int(which[1])
                w = (wq if kind == "q" else wk)[s]
                x = st["xq"] if kind == "q" else st["xkv"]
                ps = pool.tile([128, N], f32, tag=pool.name.replace("ps_", "ps"))
                nc.tensor.matmul(ps[:], w, x)
                sb = bpool.tile([128, N], bf16, tag=which)
                DR[DRAIN_SCHED[(which, b)]](sb[:], ps[:])
                (st["qs"] if kind == "q" else st["ks"])[s] = sb

            def proj_v(b, st, pool):
                psv = pool.tile([128, NT * H * HDA], f32, tag=pool.name.replace("ps_", "ps"))
                for t in range(NT):
                    nc.tensor.matmul(
                        psv[:, t * H * HDA:(t + 1) * H * HDA],
                        st["xkv"][0:DIM, t * 128:(t + 1) * 128], wv[0:DIM, :],
                        start=(t == 0), stop=(t == NT - 1),
                        skip_group_check=True)
                va = bpool.tile([128, NT, H * HDA], bf16, tag="va")
                DR[DRAIN_SCHED[("va", b)]](
                    va[:].rearrange("p t c -> p (t c)"), psv[:])
                # AP-scalar operands must be f32; convert the mask column
                # here (not in dma_in) so it doesn't head the DVE queue
                # waiting on the adjacency DMA while the k drains starve
                mcol = spool.tile([128, NT], f32, tag="mcol")
                nc.vector.tensor_copy(mcol[:], st["adjm"][:, :, N])
                st["mcol"] = mcol
                vm = bpool.tile([128, NT, H * DH], bf16, tag="vm")
                for t in range(NT):
                    nc.gpsimd.tensor_tensor(
                        vm[:, t, :].rearrange("p (h c) -> p h c", c=DH),
                        va[:, t, :].rearrange("p (h c) -> p h c", c=HDA)[:, :, 0:DH],
                        st["mcol"][:, t:t + 1].unsqueeze(2).broadcast_to(
                            [128, H, DH]),
                        op=OP.mult)
                nc.gpsimd.memset(
                    va[:].rearrange("p t (h c) -> p t h c", c=HDA)[:, :, :, DH:HDA],
                    1.0)
                st["va"], st["vm"] = va, vm

            def emit_dots(b, jt, st, cis=(0, 1, 2), pts=None):
                """Three 2-head chunks for token tile jt; chunk ci owns
                pool ci (ACT: P0/P1, DVE: P2) so each engine's next chunk
                is always pre-fillable."""
                jsl = slice(jt * 128, (jt + 1) * 128)
                if pts is None:
                    pts = []
                for ci in cis:
                    psd = PSD[ci].tile([128, 2, N], f32, tag=f"psd{ci}")
                    for k in range(2):
                        h = 2 * ci + k
                        t, g = divmod(h, HPP)
                        c = 32 * g
                        nc.tensor.matmul(
                            psd[:, k, :], st["ks"][t][c:c + DH + 1, jsl],
                            st["qs"][t][c:c + DH + 1, :],
                            tile_position=(c, 0))
                    eng = EXP_SCHED[(b, jt, ci)]
                    if eng == "act":
                        pt = ptp.tile([128, 2, N], bf16, tag=f"pt{ci}")
                        nc.scalar.activation(
                            pt[:].rearrange("p h n -> p (h n)"),
                            psd[:].rearrange("p h n -> p (h n)"), AF.Exp)
                    else:
                        pt = ptp.tile([128, 2, N], i16, tag=f"pt{ci}")
                        nc.vector.tensor_scalar(
                            pt[:].rearrange("p h n -> p (h n)"),
                            psd[:].rearrange("p h n -> p (h n)"),
                            FE_A, FE_B, op0=OP.mult, op1=OP.add)
                    pts.append(pt)
                return pts

            def emit_av_jt(b, st, psos, pts_jt, jt, parts=("adj", 0, 1, 2)):
                """Accumulate this jt's attention@V / adj@vm contributions
                into the two it-pair psum banks. `parts` selects which
                chunk's head-matmuls (0/1/2) or the adjacency ("adj") are
                emitted, so each piece only queues behind the exp it needs.
                The psum group start is the first jt0 adj matmul and the
                stop is the last jt3 ci2 matmul (same-engine in-order
                execution keeps the zeroing first and the stop last)."""
                for part in parts:
                    for itp in range(2):
                        pso = psos[itp]
                        for ki in range(2):
                            it = 2 * itp + ki
                            isl = slice(it * 128, (it + 1) * 128)
                            if part == "adj":
                                nc.tensor.matmul(
                                    pso[:, ki, H * HDA:],
                                    st["adjs"][:, jt, isl],
                                    st["vm"][:, jt, :],
                                    start=(jt == 0 and ki == 0),
                                    stop=False, skip_group_check=True)
                                continue
                            for hh in range(2):
                                h = 2 * part + hh
                                pt = pts_jt[part]
                                ptv = pt[:, hh, isl]
                                if pt.dtype != bf16:
                                    ptv = ptv.bitcast(bf16)
                                nc.tensor.matmul(
                                    pso[:, ki, h * HDA:(h + 1) * HDA], ptv,
                                    st["va"][:, jt, h * HDA:(h + 1) * HDA],
                                    start=False,
                                    stop=(jt == NT - 1 and part == 2
                                          and ki == 1 and hh == 1),
                                    skip_group_check=True)

            def emit_post(b, st, psos, attn):
                """drain AV psum; softmax divide (Pool) + adj add -> attn."""
                for itp in range(2):
                    pso = psos[itp]
                    pv = opool.tile([128, 2, H * HDA + H * DH], bf16,
                                    tag=f"pv{itp}")
                    DR[DRAIN_SCHED[(f"pv{itp}", b)]](
                        pv[:].rearrange("p k c -> p (k c)"),
                        pso[:].rearrange("p k c -> p (k c)"))
                    pvh = pv[:, :, 0:H * HDA].rearrange(
                        "p k (h c) -> p k h c", c=HDA)
                    rs = spool.tile([128, 2, H], f32, tag=f"rs{itp}")
                    nc.vector.reciprocal(
                        rs[:].unsqueeze(3), pvh[:, :, :, DH:HDA])
                    t1 = spool.tile([128, 2, H * DH], bf16, tag=f"t1{itp}")
                    eng = (nc.gpsimd if itp == 0 else nc.vector)
                    eng.tensor_tensor(
                        t1[:].rearrange("p k (h c) -> p k h c", c=DH),
                        pvh[:, :, :, 0:DH],
                        rs[:].unsqueeze(3).broadcast_to([128, 2, H, DH]),
                        op=OP.mult)
                    for ki in range(2):
                        it = 2 * itp + ki
                        nc.vector.scalar_tensor_tensor(
                            attn[:, it, 0:H * DH],
                            pv[:, ki, H * HDA:], st["mcol"][:, it:it + 1],
                            t1[:, ki, :], op0=OP.mult, op1=OP.add)

            def emit_out(b, st, attn):
                """transpose + output projection + store, all 4 its."""
                outT = bpool.tile([79, N], bf16, tag="outT")
                ysb = opool.tile([128, NT, DIM], bf16, tag="ysb")
                # psy lives in PSD1; psa ping-pongs PSD0/PSD2 (no shared
                # pool, so interleaving transposes and psy matmuls is safe)
                psy = PSD[1].tile([128, NT * DIM], f32, tag="psd1")
                for it in range(NT):
                    isl = slice(it * 128, (it + 1) * 128)
                    pool = PSD[2 * (it % 2)]
                    psa = pool.tile([79, 128], bf16,
                                    tag=pool.name.replace("ps_", "ps"))
                    nc.tensor.transpose(
                        psa[:], attn[:, it, 0:79], identity[:])
                    DR[DRAIN_SCHED[("outT", b)]](outT[:, isl], psa[:])
                    # two psum groups (its 0-1 / 2-3) in one bank: the
                    # second group's start=False rides the zeroing done by
                    # the first, so each half can be drained and DMA'd as
                    # soon as its own stop lands (overlaps the final DMA
                    # setup with the second half's compute)
                    nc.tensor.matmul(
                        psy[:, it * DIM:(it + 1) * DIM],
                        outT[:, isl], wo[:],
                        start=(it == 0), stop=(it in (1, NT - 1)),
                        skip_group_check=True)
                    if b == BPC - 1 and it in (1, NT - 1):
                        half = slice(0, 2) if it == 1 else slice(2, NT)
                        DR[DRAIN_SCHED[("ysb", b)]](
                            ysb[:, half, :].rearrange("p t f -> p (t f)"),
                            psy[:, half.start * DIM:half.stop * DIM])
                        nc.sync.dma_start(
                            yout[b].rearrange("(t p) f -> p t f", p=128)
                            [:, half, :], ysb[:, half, :])
                if b != BPC - 1:
                    DR[DRAIN_SCHED[("ysb", b)]](
                        ysb[:].rearrange("p t f -> p (t f)"), psy[:])
                    nc.sync.dma_start(
                        yout[b].rearrange("(t p) f -> p t f", p=128), ysb[:])

            def new_attn():
                attn = bpool.tile([128, NT, ATTN_W], bf16, tag="attn")
                nc.gpsimd.memset(attn[:, :, DIM:ATTN_W], 1.0)
                return attn

            def new_psos():
                return [PSA[itp].tile([128, 2, H * HDA + H * DH], f32,
                                      tag=f"av{itp}", name=f"pso{itp}")
                        for itp in range(2)]

            # ---- emission schedule (per-engine queues execute in this
            # order; chosen so the exp streams never sit behind later-dep
            # work in their queues) ----
            st0 = dma_in(0)
            proj_qk(0, st0, "k0", PSD[1])
            proj_qk(0, st0, "q0", PSD[0])
            proj_qk(0, st0, "q1", PSD[2])
            psos0 = new_psos()
            attn0 = new_attn()
            p00 = emit_dots(0, 0, st0, cis=(0,))
            proj_qk(0, st0, "k1", PSD[1])
            emit_dots(0, 0, st0, cis=(1,), pts=p00)
            proj_v(0, st0, PSD[2])
            emit_dots(0, 0, st0, cis=(2,), pts=p00)
            p01 = emit_dots(0, 1, st0)
            emit_av_jt(0, st0, psos0, p00, 0, parts=("adj", 0, 1))
            st1 = dma_in(1)
            proj_qk(1, st1, "q0", PSD[0])
            proj_qk(1, st1, "k0", PSD[1])
            proj_qk(1, st1, "q1", PSD[2])
            p02 = emit_dots(0, 2, st0)
            emit_av_jt(0, st0, psos0, p00, 0, parts=(2,))
            emit_av_jt(0, st0, psos0, p01, 1, parts=("adj", 0, 1))
            proj_qk(1, st1, "k1", PSD[0])
            proj_v(1, st1, PSD[1])
            p03 = emit_dots(0, 3, st0)
            emit_av_jt(0, st0, psos0, p01, 1, parts=(2,))
            emit_av_jt(0, st0, psos0, p02, 2, parts=("adj", 0, 1))
            pts1 = [emit_dots(1, 0, st1)]
            emit_av_jt(0, st0, psos0, p02, 2, parts=(2,))
            emit_av_jt(0, st0, psos0, p03, 3, parts=("adj", 0, 1))
            pts1.append(emit_dots(1, 1, st1))
            emit_av_jt(0, st0, psos0, p03, 3, parts=(2,))
            emit_post(0, st0, psos0, attn0)
            pts1.append(emit_dots(1, 2, st1))
            psos1 = new_psos()
            attn1 = new_attn()
            emit_av_jt(1, st1, psos1, pts1[0], 0)
            emit_av_jt(1, st1, psos1, pts1[1], 1, parts=("adj", 0, 1))
            pts1.append(emit_dots(1, 3, st1))
            emit_av_jt(1, st1, psos1, pts1[1], 1, parts=(2,))
            emit_av_jt(1, st1, psos1, pts1[2], 2)
            emit_out(0, st0, attn0)
            emit_av_jt(1, st1, psos1, pts1[3], 3)
            emit_post(1, st1, psos1, attn1)
            emit_out(1, st1, attn1)

    if walrus_patches:
        _split_waits(nc, mybir)
    return nc


def _prep_inputs(x, mask, adjacency_mat, W_qkv, W_out, b_out):
    x = np.asarray(x, np.float32)
    maskf = np.ascontiguousarray(np.asarray(mask, np.float32))
    adj = np.asarray(adjacency_mat, np.float32)
    wall = _host_weights(
        np.asarray(W_qkv, np.float32), np.asarray(W_out, np.float32),
        np.asarray(b_out, np.float32))
    xt = x.transpose(0, 2, 1)                      # [B, DIM, N]
    # xall: [xq | xkv] side by side (one DMA per batch)
    xall = np.zeros((B, 80, 2 * N), np.float32)
    xall[:, 0:DIM, 0:N] = xt * maskf[:, None, :]   # pre-masked x^T
    xall[:, 79, 0:N] = maskf
    xall[:, 0:DIM, N:2 * N] = xt
    xall[:, 78, N:2 * N] = maskf
    xall[:, 79, N:2 * N] = 1.0
    xall = xall.astype(ml_dtypes.bfloat16)
    wallf = np.asarray(wall)
    # adjm: per 128-row tile, [adj^T * 0.5 tile cols | mask col]
    adjt = (adj * LG).transpose(0, 2, 1)           # [B, j, i]
    adjm = np.zeros((B, 128, NT, N + 1), np.float32)
    adjm[:, :, :, 0:N] = adjt.reshape(B, NT, 128, N).transpose(0, 2, 1, 3)
    adjm[:, :, :, N] = maskf.reshape(B, NT, 128).transpose(0, 2, 1)
    adjm = adjm.reshape(B, 128, NT * (N + 1)).astype(ml_dtypes.bfloat16)
    in_maps = []
    for c in range(NCORES):
        s = slice(c * BPC, (c + 1) * BPC)
        b0 = c * BPC
        in_maps.append({
            "xw0": np.ascontiguousarray(
                np.concatenate([xall[b0], wallf], axis=1)),
            "xall1": np.ascontiguousarray(xall[b0 + 1]),
            "adjm": np.ascontiguousarray(adjm[s]),
        })
    return in_maps


LAST_EXEC_NS = None
LAST_RESULT = None


def kernel(x, mask, adjacency_mat, W_qkv, W_out, b_out):
    global LAST_EXEC_NS, LAST_RESULT
    from concourse.bass_utils import run_bass_kernel_spmd

    if "nc" not in _CACHE:
        _CACHE["nc"] = _build_bass()
    nc = _CACHE["nc"]

    in_maps = _prep_inputs(x, mask, adjacency_mat, W_qkv, W_out, b_out)
    trace = bool(int(os.environ.get("KERNEL_TRACE", "0")))
    res = run_bass_kernel_spmd(
        nc, in_maps, core_ids=list(range(NCORES)), trace=trace)
    LAST_EXEC_NS = res.exec_time_ns
    LAST_RESULT = res
    y = np.concatenate(
        [np.asarray(res.results[c]["yout"]).astype(np.float32)
         for c in range(NCORES)], axis=0)
    return np.ascontiguousarray(y)
